# revision 6
# baseline (speedup 1.0000x reference)
"""Trainium2 kernel for nn_AttentionFusion (dense_transformer).

Math: the reference MHA has seq_len 1 for q and kv, so softmax over the
single kv position is identically 1.0 and the attention output equals the
value projection. The whole module therefore collapses (exactly, up to fp
rounding) to one affine map per input stream:

    out = relu(audio @ Waa.T + visual @ Wva.T + b)

with
    Wvo = Wo @ Wi[2E:]             bvo = Wo @ bi[2E:] + bo
    Wfv = Wf[:, :E] @ Wvo          Wfa = Wf[:, E:] @ Wvo
    Waa = Wfa @ Wa                 Wva = Wfv @ Wv
    b   = Wfa @ ba + Wfv @ bv + (Wf[:, :E] + Wf[:, E:]) @ bvo + bf

Weight composition is done on host in float64 (cheap: ~15 GFLOP), the big
GEMM (32768 x 4096 @ 4096 x 1024, 275 GFLOP) runs on 8 NeuronCores, batch
sharded (pure data parallel per the sharding hint).

Device layout per core (operands bf16 on the PE: 1 cyc/row like f32r but
half the DMA/SBUF traffic; measured rel-err 2.1e-3 vs the 2e-2 gate):
    xt  [K=4096, BC=4096]  bf16 - per-core activations, feature-major
    wt  [K=4096, E=1024]   bf16 - composed weight, feature-major (replicated)
    bias[P=128,  E=1024]   f32  - row-replicated bias
    out [BC=4096, E=1024]  f32  - natural layout

PSUM tile [128 batch, 512 outfeat] (one bank; matmul cannot cross a PSUM
bank boundary, so 512 is the max out free dim): stationary = xt subtile
[128k, 128b] (reused across the 2 outfeat halves), moving = wt tile
[128k, 512e]. Epilogue: DVE adds the row-replicated bias PSUM->SBUF,
ScalarE applies Relu, DMA out in natural layout.

DMA preamble is ordered just-in-time as (xch[k], wt[k]) pairs so the PE
starts after ~0.4 MB instead of after the whole weight set; with bf16 the
steady per-k DMA demand (384 KB / 1.2 us) stays under the PE's k-step time
(1.7 us), so the first k-sweep is never starved. The last two batch tiles
are 256 rows instead of one 512-row tile so the final PSUM drain +
store-out tail is halved.
"""

import os
import sys

import numpy as np

sys.path.insert(0, "/opt/trn_rl_repo")

import ml_dtypes

import concourse.bacc as bacc
import concourse.mybir as mybir
import concourse.tile as tile
from concourse.bass_utils import run_bass_kernel_spmd


def _ensure_ntff_hook():
    """Register the axon NTFF profile hook if boot() couldn't (the image's
    antenv may lack axon_hooks; without this, trace=True silently degrades)."""
    try:
        import antenv.axon_hooks as ah
    except ImportError:
        import types

        import antenv

        ah = types.ModuleType("antenv.axon_hooks")
        ah._HOOK = None
        ah.set_axon_ntff_profile_hook = lambda h: setattr(ah, "_HOOK", h)
        ah.get_axon_ntff_profile_hook = lambda: ah._HOOK
        sys.modules["antenv.axon_hooks"] = ah
        antenv.axon_hooks = ah
    try:
        if ah.get_axon_ntff_profile_hook() is None:
            from trn_agent_boot.trn_boot import _ntff_profile_via_ctypes

            ah.set_axon_ntff_profile_hook(
                _ntff_profile_via_ctypes("/opt/axon/libaxon_pjrt.so")
            )
    except Exception:
        pass


_ensure_ntff_hook()

N_CORES = 8
B = 32768
BC = B // N_CORES  # 4096 batch rows per core
K = 4096           # 2048 audio + 2048 visual features
E = 1024
P = 128

KO = K // P        # 32 contraction tiles
NB = 512           # main batch tile
# Progressively smaller final tiles shrink the end-of-kernel drain tail
# (the last tile's PSUM drain + store-out cannot overlap any compute).
TILES = [NB] * 7 + [256, 128, 128]
assert sum(TILES) == BC
M2 = E // NB       # 2 outfeat halves (PSUM free dim limit: one 2KB bank)

WARM_N = int(os.environ.get("KMM_WARM_N", "52"))  # HAM warmup matmuls
WARM_F = 64                                       # their moving free dim
RELU_ENGINE = os.environ.get("KMM_RELU", "dve")

DT_NAME = os.environ.get("KMM_DTYPE", "bf16")

_NC_CACHE = {}
LAST_RESULTS = None  # stashed BassKernelResults for test.py introspection

# Note: walrus's --enable-ldw-opt=true was tested (dedupes the shared-lhsT
# LDWEIGHTS pairs, 2048 -> 1087) but measured SLOWER: the standalone-LW form
# loses the fused matmul's background weight-buffer pipelining (+12us PE).


def _build_nc(dt_name):
    mm_dt = {
        "f32": mybir.dt.float32,
        "f32r": mybir.dt.float32r,
        "bf16": mybir.dt.bfloat16,
    }[dt_name]
    f32 = mybir.dt.float32

    nc = bacc.Bacc("TRN2", debug=False, target_bir_lowering=False)
    xt = nc.dram_tensor("xt", [K, BC], mm_dt, kind="ExternalInput").ap()
    wt = nc.dram_tensor("wt", [K, E], mm_dt, kind="ExternalInput").ap()
    bias = nc.dram_tensor("bias", [P, E], f32, kind="ExternalInput").ap()
    out = nc.dram_tensor("out", [BC, E], f32, kind="ExternalOutput").ap()

    with tile.TileContext(nc) as tc:
        with (
            tc.tile_pool(name="wpool", bufs=1) as wpool,
            tc.tile_pool(name="xpool", bufs=12) as xpool,
            tc.tile_pool(name="opool", bufs=8) as opool,
            tc.tile_pool(name="pspool", bufs=8, space="PSUM") as pspool,
        ):
            # The DMA path is one FIFO queue fanned over 16 engines (~300-346
            # GB/s measured; bigger transfers amortize per-DMA overhead):
            # arrival order == emission order. Emit (xch[k], wt[k]) pairs
            # just-in-time for batch tile 0's k-sweep: the PE can issue its
            # first matmul after one 384 KB pair instead of waiting on the
            # whole weight set.
            wt_sb = wpool.tile([P, KO, E], mm_dt)
            wt_r = wt.rearrange("(ko ki) e -> ki ko e", ki=P)
            bias_sb = wpool.tile([P, E], f32)

            # HAM warmup: the PE clock-gate defaults to 1.2 GHz and only
            # reaches 2.4 GHz after ~3.4us of sustained activity. Burn that
            # window on dummy matmuls during the DMA preamble (PE is
            # otherwise idle for ~8us waiting on the first transfers).
            if WARM_N:
                warm = wpool.tile([P, P], mm_dt)
                nc.vector.memset(warm, 1.0)
                wps = pspool.tile([P, NB], f32, tag="ps", name="ps_warm")
                for i in range(WARM_N):
                    nc.tensor.matmul(
                        wps[:, 0:WARM_F], lhsT=warm, rhs=warm[:, 0:WARM_F],
                        start=True, stop=True,
                    )

            xch0 = {}
            for k in range(8):
                xch = xpool.tile([P, NB], mm_dt, tag="xch")
                if k == 0:
                    # Split the first pair so the first matmul's operands
                    # (stationary xch[:, 0:128], moving wt[:, 0, 0:512])
                    # land after ~160 KB instead of 384 KB.
                    nc.sync.dma_start(xch[:, 0:P], xt[0:P, 0:P])
                    nc.sync.dma_start(wt_sb[:, 0, 0:NB], wt_r[:, 0, 0:NB])
                    nc.sync.dma_start(xch[:, P:NB], xt[0:P, P:NB])
                    nc.sync.dma_start(wt_sb[:, 0, NB:E], wt_r[:, 0, NB:E])
                else:
                    nc.sync.dma_start(xch, xt[k * P : (k + 1) * P, 0:NB])
                    nc.sync.dma_start(wt_sb[:, k], wt_r[:, k])
                xch0[k] = xch
            for k in range(8, KO):
                xch = xpool.tile([P, NB], mm_dt, tag="xch")
                nc.sync.dma_start(xch, xt[k * P : (k + 1) * P, 0:NB])
                xch0[k] = xch
                if k % 4 == 0:
                    # 4-ko weight chunks: fewer, larger transfers raise
                    # effective DMA bandwidth once the PE is streaming.
                    nc.sync.dma_start(wt_sb[:, k : k + 4], wt_r[:, k : k + 4])
                if k == 11:
                    # Bias early enough for the first tile's drains but off
                    # the first weight chunks' critical path.
                    nc.sync.dma_start(bias_sb, bias)

            off = 0
            for n, nb in enumerate(TILES):
                b4 = nb // P
                psums = [
                    pspool.tile([P, NB], f32, tag="ps", name=f"ps_{n}_{j}")
                    for j in range(b4 * M2)
                ]
                for k in range(KO):
                    if n == 0:
                        xch = xch0[k]
                    else:
                        xch = xpool.tile([P, nb], mm_dt, tag=f"xch{nb}")
                        nc.sync.dma_start(
                            xch, xt[k * P : (k + 1) * P, off : off + nb]
                        )
                    for b in range(b4):
                        for m in range(M2):
                            nc.tensor.matmul(
                                psums[b * M2 + m],
                                lhsT=xch[:, b * P : (b + 1) * P],
                                rhs=wt_sb[:, k, m * NB : (m + 1) * NB],
                                start=(k == 0),
                                stop=(k == KO - 1),
                            )
                for b in range(b4):
                    for m in range(M2):
                        ps = psums[b * M2 + m]
                        osb = opool.tile([P, NB], f32, tag="osb")
                        nc.vector.tensor_add(
                            out=osb,
                            in0=ps,
                            in1=bias_sb[:, m * NB : (m + 1) * NB],
                        )
                        if RELU_ENGINE == "dve":
                            # Keeping the ScalarE out of the kernel avoids
                            # its 16 KB activation-table DMA in the
                            # end-of-kernel barrier (~5us of tail).
                            nc.vector.tensor_scalar_max(osb, osb, 0.0)
                        else:
                            nc.scalar.activation(
                                osb, osb, mybir.ActivationFunctionType.Relu
                            )
                        nc.sync.dma_start(
                            out[
                                off + b * P : off + (b + 1) * P,
                                m * NB : (m + 1) * NB,
                            ],
                            osb,
                        )
                off += nb

    nc.compile()
    return nc


def _get_nc(dt_name):
    if dt_name not in _NC_CACHE:
        _NC_CACHE[dt_name] = _build_nc(dt_name)
    return _NC_CACHE[dt_name]


def _compose_weights(Wa, ba, Wv, bv, Wi, bi, Wo, bo, Wf, bf):
    f6 = lambda x: np.asarray(x, dtype=np.float64)
    Wvo = f6(Wo) @ f6(Wi[2 * E :])
    bvo = f6(Wo) @ f6(bi[2 * E :]) + f6(bo)
    Wf1, Wf2 = f6(Wf[:, :E]), f6(Wf[:, E:])
    Wfv = Wf1 @ Wvo  # applied to visual_e for audio_att
    Wfa = Wf2 @ Wvo  # applied to audio_e for visual_att
    Waa = Wfa @ f6(Wa)  # [E, 2048] applied to audio
    Wva = Wfv @ f6(Wv)  # [E, 2048] applied to visual
    b = Wfa @ f6(ba) + Wfv @ f6(bv) + (Wf1 + Wf2) @ bvo + f6(bf)
    wt = np.ascontiguousarray(
        np.concatenate([Waa, Wva], axis=1).T, dtype=np.float32
    )  # [K, E]
    return wt, b.astype(np.float32)


def kernel(audio, visual, Wa, ba, Wv, bv, Wi, bi, Wo, bo, Wf, bf):
    global LAST_RESULTS
    wt, bias = _compose_weights(Wa, ba, Wv, bv, Wi, bi, Wo, bo, Wf, bf)
    bias_bc = np.ascontiguousarray(np.broadcast_to(bias, (P, E)), np.float32)

    np_dt = {
        "f32": np.float32,
        "f32r": np.float32,
        "bf16": ml_dtypes.bfloat16,
    }[DT_NAME]
    wt = wt.astype(np_dt)
    audio = np.asarray(audio, dtype=np.float32).astype(np_dt)
    visual = np.asarray(visual, dtype=np.float32).astype(np_dt)

    in_maps = []
    for c in range(N_CORES):
        rows = slice(c * BC, (c + 1) * BC)
        xt_c = np.empty((K, BC), np_dt)
        xt_c[: K // 2] = audio[rows].T
        xt_c[K // 2 :] = visual[rows].T
        in_maps.append({"xt": xt_c, "wt": wt, "bias": bias_bc})

    nc = _get_nc(DT_NAME)
    trace = os.environ.get("KMM_TRACE", "0") == "1"
    kwargs = {}
    if os.environ.get("KMM_TRACE_ALL", "0") == "1":
        kwargs["trace_cores"] = list(range(N_CORES))
    res = run_bass_kernel_spmd(
        nc, in_maps, core_ids=list(range(N_CORES)), trace=trace, **kwargs
    )
    LAST_RESULTS = res
    out = np.concatenate([r["out"] for r in res.results], axis=0)
    return np.ascontiguousarray(out, dtype=np.float32)


# revision 8
# speedup vs baseline: 1.3514x; 1.3514x over previous
"""Trainium2 kernel for nn_AttentionFusion (dense_transformer).

Math: the reference MHA has seq_len 1 for q and kv, so softmax over the
single kv position is identically 1.0 and the attention output equals the
value projection. The whole module therefore collapses (exactly, up to fp
rounding) to one affine map per input stream:

    out = relu(audio @ Waa.T + visual @ Wva.T + b)

with
    Wvo = Wo @ Wi[2E:]             bvo = Wo @ bi[2E:] + bo
    Wfv = Wf[:, :E] @ Wvo          Wfa = Wf[:, E:] @ Wvo
    Waa = Wfa @ Wa                 Wva = Wfv @ Wv
    b   = Wfa @ ba + Wfv @ bv + (Wf[:, :E] + Wf[:, E:]) @ bvo + bf

Weight composition is done on host in float64 (cheap: ~15 GFLOP), the big
GEMM (32768 x 4096 @ 4096 x 1024, 275 GFLOP) runs on 8 NeuronCores, batch
sharded (pure data parallel per the sharding hint).

Mixed-precision contraction: the PE runs bf16 at 1 cyc/row and fp8-e4m3
DoubleRow at 0.5 cyc/row (contracting 256 rows per instruction). Putting
the last K8=1024 of the 4096 contraction rows in fp8 cuts PE time 12.5%
while the measured end-to-end max-rel error stays at 1.75e-2 vs the 2e-2
gate (bf16-only is 2.0e-3). The fp8 operands are reciprocally pre-scaled
on host (x/2^5, w*2^5) so their products land at the correct scale and
accumulate into the same PSUM group as the bf16 part; e4m3 cannot
represent the raw w ~ 1e-3 values (subnormal cutoff 2^-6) without this.

Device layout per core:
    xtb [KB=3072, BC=4096] bf16 - activations, feature-major
    xt8 [K8=1024, BC=4096] f8e4 - last K-slice, pre-scaled 2^-5
    wtb [KB, E=1024]       bf16 - composed weight, feature-major
    wt8 [K8, E]            f8e4 - pre-scaled 2^+5 (replicated)
    bias[P=128,  E]        f32  - row-replicated bias
    out [BC, E]            f32  - natural layout

PSUM tile [128 batch, 512 outfeat] (one bank; matmul cannot cross a PSUM
bank boundary): stationary = x subtile, moving = w tile. Per batch tile:
24 bf16 k-steps then 4 DoubleRow steps (lhsT [128,2,128], rhs [128,2,512])
accumulate, then DVE adds bias PSUM->SBUF, ScalarE applies Relu, DMA out.

DMA preamble is ordered just-in-time as (xch[k], wt[k]) pairs so the PE
starts after ~0.4 MB instead of after the whole weight set; bf16 per-k
demand (384 KB / 1.2 us) stays under the PE k-step time (1.7 us) so the
first sweep never starves, and the fp8 chunks ride in the slack before
the sweep reaches them. The final batch tiles shrink (512x7, 256, 128,
128) because the last tile's PSUM drain + store-out cannot overlap any
compute.
"""

import os
import sys

import numpy as np

sys.path.insert(0, "/opt/trn_rl_repo")

import ml_dtypes

import concourse.bacc as bacc
import concourse.mybir as mybir
import concourse.tile as tile
from concourse.bass_utils import run_bass_kernel_spmd


def _ensure_ntff_hook():
    """Register the axon NTFF profile hook if boot() couldn't (the image's
    antenv may lack axon_hooks; without this, trace=True silently degrades)."""
    try:
        import antenv.axon_hooks as ah
    except ImportError:
        import types

        import antenv

        ah = types.ModuleType("antenv.axon_hooks")
        ah._HOOK = None
        ah.set_axon_ntff_profile_hook = lambda h: setattr(ah, "_HOOK", h)
        ah.get_axon_ntff_profile_hook = lambda: ah._HOOK
        sys.modules["antenv.axon_hooks"] = ah
        antenv.axon_hooks = ah
    try:
        if ah.get_axon_ntff_profile_hook() is None:
            from trn_agent_boot.trn_boot import _ntff_profile_via_ctypes

            ah.set_axon_ntff_profile_hook(
                _ntff_profile_via_ctypes("/opt/axon/libaxon_pjrt.so")
            )
    except Exception:
        pass


_ensure_ntff_hook()

N_CORES = 8
B = 32768
BC = B // N_CORES  # 4096 batch rows per core
K = 4096           # 2048 audio + 2048 visual features
E = 1024
P = 128

MODE = os.environ.get("KMM_MODE", "mix8")  # "mix8" | "bf16"
K8 = 1024 if MODE == "mix8" else 0  # fp8 contraction rows (last K-slice)
A8 = 5                              # reciprocal power-of-2 operand scale
KB = K - K8
KBT = KB // P      # bf16 contraction tiles
J8 = K8 // (2 * P) # fp8 DoubleRow steps (256 rows each)
NB = 512           # main batch tile
# Progressively smaller final tiles shrink the end-of-kernel drain tail.
TILES = [NB] * 7 + [256, 128, 128]
assert sum(TILES) == BC
M2 = E // NB       # 2 outfeat halves (PSUM free dim limit: one 2KB bank)

_NC_CACHE = {}
LAST_RESULTS = None  # stashed BassKernelResults for test.py introspection


def _build_nc(mode):
    bf16 = mybir.dt.bfloat16
    f8 = mybir.dt.float8e4
    f32 = mybir.dt.float32

    nc = bacc.Bacc("TRN2", debug=False, target_bir_lowering=False)
    xtb = nc.dram_tensor("xtb", [KB, BC], bf16, kind="ExternalInput").ap()
    wtb = nc.dram_tensor("wtb", [KB, E], bf16, kind="ExternalInput").ap()
    if K8:
        xt8 = nc.dram_tensor("xt8", [K8, BC], f8, kind="ExternalInput").ap()
        wt8 = nc.dram_tensor("wt8", [K8, E], f8, kind="ExternalInput").ap()
    bias = nc.dram_tensor("bias", [P, E], f32, kind="ExternalInput").ap()
    out = nc.dram_tensor("out", [BC, E], f32, kind="ExternalOutput").ap()

    with tile.TileContext(nc) as tc:
        with (
            tc.tile_pool(name="wpool", bufs=1) as wpool,
            tc.tile_pool(name="xpool", bufs=12) as xpool,
            tc.tile_pool(name="x8pool", bufs=6) as x8pool,
            tc.tile_pool(name="opool", bufs=8) as opool,
            tc.tile_pool(name="pspool", bufs=8, space="PSUM") as pspool,
        ):
            # DMA arrival order == emission order (one FIFO fanned over the
            # engines). Emit (xch[k], wt[k]) pairs just-in-time for batch
            # tile 0's k-sweep.
            wtb_sb = wpool.tile([P, KBT, E], bf16)
            wtb_r = wtb.rearrange("(ko ki) e -> ki ko e", ki=P)
            if K8:
                wt8_sb = wpool.tile([P, 2 * J8, E], f8)
                wt8_r = wt8.rearrange("(ko ki) e -> ki ko e", ki=P)
            bias_sb = wpool.tile([P, E], f32)

            xch0 = {}
            for k in range(8):
                xch = xpool.tile([P, NB], bf16, tag="xch")
                if k == 0:
                    # Split the first pair so the first matmul's operands
                    # (stationary xch[:, 0:128], moving wt[:, 0, 0:512])
                    # land after ~160 KB instead of 384 KB.
                    nc.sync.dma_start(xch[:, 0:P], xtb[0:P, 0:P])
                    nc.sync.dma_start(wtb_sb[:, 0, 0:NB], wtb_r[:, 0, 0:NB])
                    nc.sync.dma_start(xch[:, P:NB], xtb[0:P, P:NB])
                    nc.sync.dma_start(wtb_sb[:, 0, NB:E], wtb_r[:, 0, NB:E])
                else:
                    nc.sync.dma_start(xch, xtb[k * P : (k + 1) * P, 0:NB])
                    nc.sync.dma_start(wtb_sb[:, k], wtb_r[:, k])
                xch0[k] = xch
            for k in range(8, KBT):
                xch = xpool.tile([P, NB], bf16, tag="xch")
                nc.sync.dma_start(xch, xtb[k * P : (k + 1) * P, 0:NB])
                xch0[k] = xch
                if k % 4 == 0:
                    nc.sync.dma_start(wtb_sb[:, k : k + 4], wtb_r[:, k : k + 4])
                if k == 11:
                    nc.sync.dma_start(bias_sb, bias)
            xch80 = {}
            for j in range(J8):
                # fp8 chunks ride in the first sweep's DMA slack (the PE is
                # still ~17 us away from needing them when these are issued).
                xch8 = x8pool.tile([P, 2, NB], f8, tag="xch8")
                for i in range(2):
                    r = (2 * j + i) * P
                    nc.sync.dma_start(xch8[:, i], xt8[r : r + P, 0:NB])
                xch80[j] = xch8
                nc.sync.dma_start(
                    wt8_sb[:, 2 * j : 2 * j + 2], wt8_r[:, 2 * j : 2 * j + 2]
                )

            off = 0
            for n, nb in enumerate(TILES):
                b4 = nb // P
                psums = [
                    pspool.tile([P, NB], f32, tag="ps", name=f"ps_{n}_{j}")
                    for j in range(b4 * M2)
                ]
                for k in range(KBT):
                    if n == 0:
                        xch = xch0[k]
                    else:
                        xch = xpool.tile([P, nb], bf16, tag=f"xch{nb}")
                        nc.sync.dma_start(
                            xch, xtb[k * P : (k + 1) * P, off : off + nb]
                        )
                    for b in range(b4):
                        for m in range(M2):
                            nc.tensor.matmul(
                                psums[b * M2 + m],
                                lhsT=xch[:, b * P : (b + 1) * P],
                                rhs=wtb_sb[:, k, m * NB : (m + 1) * NB],
                                start=(k == 0),
                                stop=False,
                            )
                for j in range(J8):
                    if n == 0:
                        xch8 = xch80[j]
                    else:
                        xch8 = x8pool.tile([P, 2, nb], f8, tag=f"xch8{nb}")
                        for i in range(2):
                            r = (2 * j + i) * P
                            nc.sync.dma_start(
                                xch8[:, i], xt8[r : r + P, off : off + nb]
                            )
                    for b in range(b4):
                        for m in range(M2):
                            nc.tensor.matmul(
                                psums[b * M2 + m],
                                lhsT=xch8[:, :, b * P : (b + 1) * P],
                                rhs=wt8_sb[:, 2 * j : 2 * j + 2, m * NB : (m + 1) * NB],
                                start=False,
                                stop=(j == J8 - 1),
                                perf_mode=mybir.MatmulPerfMode.DoubleRow,
                            )
                for b in range(b4):
                    for m in range(M2):
                        ps = psums[b * M2 + m]
                        osb = opool.tile([P, NB], f32, tag="osb")
                        nc.vector.tensor_add(
                            out=osb,
                            in0=ps,
                            in1=bias_sb[:, m * NB : (m + 1) * NB],
                        )
                        nc.scalar.activation(
                            osb, osb, mybir.ActivationFunctionType.Relu
                        )
                        nc.sync.dma_start(
                            out[
                                off + b * P : off + (b + 1) * P,
                                m * NB : (m + 1) * NB,
                            ],
                            osb,
                        )
                off += nb

    nc.compile()
    return nc


def _get_nc(mode):
    if mode not in _NC_CACHE:
        _NC_CACHE[mode] = _build_nc(mode)
    return _NC_CACHE[mode]


def _compose_weights(Wa, ba, Wv, bv, Wi, bi, Wo, bo, Wf, bf):
    f6 = lambda x: np.asarray(x, dtype=np.float64)
    Wvo = f6(Wo) @ f6(Wi[2 * E :])
    bvo = f6(Wo) @ f6(bi[2 * E :]) + f6(bo)
    Wf1, Wf2 = f6(Wf[:, :E]), f6(Wf[:, E:])
    Wfv = Wf1 @ Wvo  # applied to visual_e for audio_att
    Wfa = Wf2 @ Wvo  # applied to audio_e for visual_att
    Waa = Wfa @ f6(Wa)  # [E, 2048] applied to audio
    Wva = Wfv @ f6(Wv)  # [E, 2048] applied to visual
    b = Wfa @ f6(ba) + Wfv @ f6(bv) + (Wf1 + Wf2) @ bvo + f6(bf)
    wt = np.concatenate([Waa, Wva], axis=1).T  # [K, E] float64
    return wt, b.astype(np.float32)


def kernel(audio, visual, Wa, ba, Wv, bv, Wi, bi, Wo, bo, Wf, bf):
    global LAST_RESULTS
    wt, bias = _compose_weights(Wa, ba, Wv, bv, Wi, bi, Wo, bo, Wf, bf)
    bias_bc = np.ascontiguousarray(np.broadcast_to(bias, (P, E)), np.float32)

    bfdt = ml_dtypes.bfloat16
    f8 = ml_dtypes.float8_e4m3
    wtb = np.ascontiguousarray(wt[:KB]).astype(bfdt)
    if K8:
        wt8 = np.ascontiguousarray(wt[KB:] * 2.0**A8).astype(f8)
    audio = np.asarray(audio, dtype=np.float32)
    visual = np.asarray(visual, dtype=np.float32)

    in_maps = []
    for c in range(N_CORES):
        rows = slice(c * BC, (c + 1) * BC)
        at = audio[rows].T  # [2048, BC]
        vt = visual[rows].T  # [2048, BC]
        xtb_c = np.empty((KB, BC), bfdt)
        xtb_c[:2048] = at
        xtb_c[2048:] = vt[: KB - 2048]
        m = {"xtb": xtb_c, "wtb": wtb, "bias": bias_bc}
        if K8:
            m["xt8"] = (vt[KB - 2048 :] * 2.0**-A8).astype(f8)
            m["wt8"] = wt8
        in_maps.append(m)

    nc = _get_nc(MODE)
    trace = os.environ.get("KMM_TRACE", "0") == "1"
    kwargs = {}
    if os.environ.get("KMM_TRACE_ALL", "0") == "1":
        kwargs["trace_cores"] = list(range(N_CORES))
    res = run_bass_kernel_spmd(
        nc, in_maps, core_ids=list(range(N_CORES)), trace=trace, **kwargs
    )
    LAST_RESULTS = res
    out = np.concatenate([r["out"] for r in res.results], axis=0)
    return np.ascontiguousarray(out, dtype=np.float32)


# revision 12
# speedup vs baseline: 1.3700x; 1.0138x over previous
"""Trainium2 kernel for nn_AttentionFusion (dense_transformer).

Math: the reference MHA has seq_len 1 for q and kv, so softmax over the
single kv position is identically 1.0 and the attention output equals the
value projection. The whole module therefore collapses (exactly, up to fp
rounding) to one affine map per input stream:

    out = relu(audio @ Waa.T + visual @ Wva.T + b)

with
    Wvo = Wo @ Wi[2E:]             bvo = Wo @ bi[2E:] + bo
    Wfv = Wf[:, :E] @ Wvo          Wfa = Wf[:, E:] @ Wvo
    Waa = Wfa @ Wa                 Wva = Wfv @ Wv
    b   = Wfa @ ba + Wfv @ bv + (Wf[:, :E] + Wf[:, E:]) @ bvo + bf

Weight composition is done on host in float64 (cheap: ~15 GFLOP), the big
GEMM (32768 x 4096 @ 4096 x 1024, 275 GFLOP) runs on 8 NeuronCores, batch
sharded (pure data parallel per the sharding hint).

Mixed-precision contraction: the PE runs bf16 at 1 cyc/row and fp8-e4m3
DoubleRow at 0.5 cyc/row (contracting 256 rows per instruction). Putting
the last K8=1024 of the 4096 contraction rows in fp8 cuts PE time 12.5%
while the measured end-to-end max-rel error stays at 1.75e-2 vs the 2e-2
gate (bf16-only is 2.0e-3). The fp8 operands are reciprocally pre-scaled
on host (x/2^5, w*2^5) so their products land at the correct scale and
accumulate into the same PSUM group as the bf16 part; e4m3 cannot
represent the raw w ~ 1e-3 values (subnormal cutoff 2^-6) without this.

Device layout per core:
    xtb [KB=3072, BC=4096] bf16 - activations, feature-major
    xt8 [K8=1024, BC=4096] f8e4 - last K-slice, pre-scaled 2^-5
    wtb [KB, E=1024]       bf16 - composed weight, feature-major
    wt8 [K8, E]            f8e4 - pre-scaled 2^+5 (replicated)
    bias[P=128,  E]        f32  - row-replicated bias
    out [BC, E]            f32  - natural layout

PSUM tile [128 batch, 512 outfeat] (one bank; matmul cannot cross a PSUM
bank boundary): stationary = x subtile, moving = w tile. Per batch tile:
24 bf16 k-steps then 4 DoubleRow steps (lhsT [128,2,128], rhs [128,2,512])
accumulate, then DVE adds bias PSUM->SBUF, ScalarE applies Relu, DMA out.

DMA preamble is ordered just-in-time as (xch[k], wt[k]) pairs so the PE
starts after ~0.4 MB instead of after the whole weight set; bf16 per-k
demand (384 KB / 1.2 us) stays under the PE k-step time (1.7 us) so the
first sweep never starves, and the fp8 chunks ride in the slack before
the sweep reaches them. The final batch tiles shrink (512x7, 256, 128,
128) because the last tile's PSUM drain + store-out cannot overlap any
compute.
"""

import os
import sys

import numpy as np

sys.path.insert(0, "/opt/trn_rl_repo")

import ml_dtypes

import concourse.bacc as bacc
import concourse.mybir as mybir
import concourse.tile as tile
from concourse.bass_utils import run_bass_kernel_spmd


def _ensure_ntff_hook():
    """Register the axon NTFF profile hook if boot() couldn't (the image's
    antenv may lack axon_hooks; without this, trace=True silently degrades)."""
    try:
        import antenv.axon_hooks as ah
    except ImportError:
        import types

        import antenv

        ah = types.ModuleType("antenv.axon_hooks")
        ah._HOOK = None
        ah.set_axon_ntff_profile_hook = lambda h: setattr(ah, "_HOOK", h)
        ah.get_axon_ntff_profile_hook = lambda: ah._HOOK
        sys.modules["antenv.axon_hooks"] = ah
        antenv.axon_hooks = ah
    try:
        if ah.get_axon_ntff_profile_hook() is None:
            from trn_agent_boot.trn_boot import _ntff_profile_via_ctypes

            ah.set_axon_ntff_profile_hook(
                _ntff_profile_via_ctypes("/opt/axon/libaxon_pjrt.so")
            )
    except Exception:
        pass


_ensure_ntff_hook()

N_CORES = 8
B = 32768
BC = B // N_CORES  # 4096 batch rows per core
K = 4096           # 2048 audio + 2048 visual features
E = 1024
P = 128

MODE = os.environ.get("KMM_MODE", "mix8")  # "mix8" | "bf16"
K8 = 1024 if MODE == "mix8" else 0  # fp8 contraction rows (last K-slice)
A8 = 5                              # reciprocal power-of-2 operand scale
KB = K - K8
KBT = KB // P      # bf16 contraction tiles
J8 = K8 // (2 * P) # fp8 DoubleRow steps (256 rows each)
NB = 512           # main batch tile
# Two 256-row final tiles shrink the end-of-kernel drain tail. No smaller:
# a tile costs ~30 DMA issues (~650 ns each on the issuing engine) and a
# 128-row tile's 12 us sweep can't cover that, so the PE starves.
TILES = [NB] * 7 + [256, 256]
assert sum(TILES) == BC
M2 = E // NB       # 2 outfeat halves (PSUM free dim limit: one 2KB bank)

_NC_CACHE = {}
LAST_RESULTS = None  # stashed BassKernelResults for test.py introspection


def _build_nc(mode):
    bf16 = mybir.dt.bfloat16
    f8 = mybir.dt.float8e4
    f32 = mybir.dt.float32

    nc = bacc.Bacc("TRN2", debug=False, target_bir_lowering=False)
    xtb = nc.dram_tensor("xtb", [KB, BC], bf16, kind="ExternalInput").ap()
    wtb = nc.dram_tensor("wtb", [KB, E], bf16, kind="ExternalInput").ap()
    if K8:
        xt8 = nc.dram_tensor("xt8", [K8, BC], f8, kind="ExternalInput").ap()
        wt8 = nc.dram_tensor("wt8", [K8, E], f8, kind="ExternalInput").ap()
    bias = nc.dram_tensor("bias", [P, E], f32, kind="ExternalInput").ap()
    out = nc.dram_tensor("out", [BC, E], f32, kind="ExternalOutput").ap()

    with tile.TileContext(nc) as tc:
        with (
            tc.tile_pool(name="wpool", bufs=1) as wpool,
            tc.tile_pool(name="xpool", bufs=12) as xpool,
            tc.tile_pool(name="x8pool", bufs=6) as x8pool,
            tc.tile_pool(name="opool", bufs=8) as opool,
            tc.tile_pool(name="pspool", bufs=8, space="PSUM") as pspool,
        ):
            # DMA arrival order == emission order per queue. Activations
            # issue from the Sync queue and weights from the GpSimd queue in
            # parallel (a single queue's ~650ns-per-issue rate would put the
            # issue stream on the critical path); per-queue FIFO keeps each
            # stream just-in-time for batch tile 0's k-sweep. Output stores
            # later issue from the Scalar queue for the same reason.
            wtb_sb = wpool.tile([P, KBT, E], bf16)
            wtb_r = wtb.rearrange("(ko ki) e -> ki ko e", ki=P)
            if K8:
                wt8_sb = wpool.tile([P, 2 * J8, E], f8)
                wt8_r = wt8.rearrange("(ko ki) e -> ki ko e", ki=P)
                xt8_r = xt8.rearrange("(c ki) b -> ki c b", ki=P)
            bias_sb = wpool.tile([P, E], f32)

            xch0 = {}
            for k in range(8):
                xch = xpool.tile([P, NB], bf16, tag="xch")
                if k == 0:
                    # Split the first pair so the first matmul's operands
                    # (stationary xch[:, 0:128], moving wt[:, 0, 0:512])
                    # land as early as possible.
                    nc.sync.dma_start(xch[:, 0:P], xtb[0:P, 0:P])
                    nc.gpsimd.dma_start(wtb_sb[:, 0, 0:NB], wtb_r[:, 0, 0:NB])
                    nc.sync.dma_start(xch[:, P:NB], xtb[0:P, P:NB])
                    nc.gpsimd.dma_start(wtb_sb[:, 0, NB:E], wtb_r[:, 0, NB:E])
                else:
                    nc.sync.dma_start(xch, xtb[k * P : (k + 1) * P, 0:NB])
                    nc.gpsimd.dma_start(wtb_sb[:, k], wtb_r[:, k])
                xch0[k] = xch
            for k in range(8, KBT):
                if k % 4 == 0:
                    nc.gpsimd.dma_start(
                        wtb_sb[:, k : k + 4], wtb_r[:, k : k + 4]
                    )
                xch = xpool.tile([P, NB], bf16, tag="xch")
                nc.sync.dma_start(xch, xtb[k * P : (k + 1) * P, 0:NB])
                xch0[k] = xch
                if k == 11:
                    nc.gpsimd.dma_start(bias_sb, bias)
            xch80 = {}
            for j in range(J8):
                # fp8 chunks ride in the first sweep's DMA slack (the PE is
                # still ~17 us away from needing them when these are issued).
                xch8 = x8pool.tile([P, 2, NB], f8, tag="xch8")
                nc.sync.dma_start(xch8, xt8_r[:, 2 * j : 2 * j + 2, 0:NB])
                xch80[j] = xch8
                nc.gpsimd.dma_start(
                    wt8_sb[:, 2 * j : 2 * j + 2], wt8_r[:, 2 * j : 2 * j + 2]
                )

            off = 0
            for n, nb in enumerate(TILES):
                b4 = nb // P
                psums = [
                    pspool.tile([P, NB], f32, tag="ps", name=f"ps_{n}_{j}")
                    for j in range(b4 * M2)
                ]
                for k in range(KBT):
                    if n == 0:
                        xch = xch0[k]
                    else:
                        xch = xpool.tile([P, nb], bf16, tag=f"xch{nb}")
                        nc.sync.dma_start(
                            xch, xtb[k * P : (k + 1) * P, off : off + nb]
                        )
                    for b in range(b4):
                        for m in range(M2):
                            nc.tensor.matmul(
                                psums[b * M2 + m],
                                lhsT=xch[:, b * P : (b + 1) * P],
                                rhs=wtb_sb[:, k, m * NB : (m + 1) * NB],
                                start=(k == 0),
                                stop=False,
                            )
                for j in range(J8):
                    if n == 0:
                        xch8 = xch80[j]
                    else:
                        xch8 = x8pool.tile([P, 2, nb], f8, tag=f"xch8{nb}")
                        nc.sync.dma_start(
                            xch8, xt8_r[:, 2 * j : 2 * j + 2, off : off + nb]
                        )
                    for b in range(b4):
                        for m in range(M2):
                            nc.tensor.matmul(
                                psums[b * M2 + m],
                                lhsT=xch8[:, :, b * P : (b + 1) * P],
                                rhs=wt8_sb[:, 2 * j : 2 * j + 2, m * NB : (m + 1) * NB],
                                start=False,
                                stop=(j == J8 - 1),
                                perf_mode=mybir.MatmulPerfMode.DoubleRow,
                            )
                for b in range(b4):
                    for m in range(M2):
                        ps = psums[b * M2 + m]
                        osb = opool.tile([P, NB], f32, tag="osb")
                        nc.vector.tensor_add(
                            out=osb,
                            in0=ps,
                            in1=bias_sb[:, m * NB : (m + 1) * NB],
                        )
                        nc.scalar.activation(
                            osb, osb, mybir.ActivationFunctionType.Relu
                        )
                        nc.scalar.dma_start(
                            out[
                                off + b * P : off + (b + 1) * P,
                                m * NB : (m + 1) * NB,
                            ],
                            osb,
                        )
                off += nb

    nc.compile()
    return nc


def _get_nc(mode):
    if mode not in _NC_CACHE:
        _NC_CACHE[mode] = _build_nc(mode)
    return _NC_CACHE[mode]


def _compose_weights(Wa, ba, Wv, bv, Wi, bi, Wo, bo, Wf, bf):
    f6 = lambda x: np.asarray(x, dtype=np.float64)
    Wvo = f6(Wo) @ f6(Wi[2 * E :])
    bvo = f6(Wo) @ f6(bi[2 * E :]) + f6(bo)
    Wf1, Wf2 = f6(Wf[:, :E]), f6(Wf[:, E:])
    Wfv = Wf1 @ Wvo  # applied to visual_e for audio_att
    Wfa = Wf2 @ Wvo  # applied to audio_e for visual_att
    Waa = Wfa @ f6(Wa)  # [E, 2048] applied to audio
    Wva = Wfv @ f6(Wv)  # [E, 2048] applied to visual
    b = Wfa @ f6(ba) + Wfv @ f6(bv) + (Wf1 + Wf2) @ bvo + f6(bf)
    wt = np.concatenate([Waa, Wva], axis=1).T  # [K, E] float64
    return wt, b.astype(np.float32)


def kernel(audio, visual, Wa, ba, Wv, bv, Wi, bi, Wo, bo, Wf, bf):
    global LAST_RESULTS
    wt, bias = _compose_weights(Wa, ba, Wv, bv, Wi, bi, Wo, bo, Wf, bf)
    bias_bc = np.ascontiguousarray(np.broadcast_to(bias, (P, E)), np.float32)

    bfdt = ml_dtypes.bfloat16
    f8 = ml_dtypes.float8_e4m3
    wtb = np.ascontiguousarray(wt[:KB]).astype(bfdt)
    if K8:
        wt8 = np.ascontiguousarray(wt[KB:] * 2.0**A8).astype(f8)
    audio = np.asarray(audio, dtype=np.float32)
    visual = np.asarray(visual, dtype=np.float32)

    in_maps = []
    for c in range(N_CORES):
        rows = slice(c * BC, (c + 1) * BC)
        at = audio[rows].T  # [2048, BC]
        vt = visual[rows].T  # [2048, BC]
        xtb_c = np.empty((KB, BC), bfdt)
        xtb_c[:2048] = at
        xtb_c[2048:] = vt[: KB - 2048]
        m = {"xtb": xtb_c, "wtb": wtb, "bias": bias_bc}
        if K8:
            m["xt8"] = (vt[KB - 2048 :] * 2.0**-A8).astype(f8)
            m["wt8"] = wt8
        in_maps.append(m)

    nc = _get_nc(MODE)
    trace = os.environ.get("KMM_TRACE", "0") == "1"
    kwargs = {}
    if os.environ.get("KMM_TRACE_ALL", "0") == "1":
        kwargs["trace_cores"] = list(range(N_CORES))
    res = run_bass_kernel_spmd(
        nc, in_maps, core_ids=list(range(N_CORES)), trace=trace, **kwargs
    )
    LAST_RESULTS = res
    out = np.concatenate([r["out"] for r in res.results], axis=0)
    return np.ascontiguousarray(out, dtype=np.float32)


# revision 13
# speedup vs baseline: 1.3883x; 1.0134x over previous
"""Trainium2 kernel for nn_AttentionFusion (dense_transformer).

Math: the reference MHA has seq_len 1 for q and kv, so softmax over the
single kv position is identically 1.0 and the attention output equals the
value projection. The whole module therefore collapses (exactly, up to fp
rounding) to one affine map per input stream:

    out = relu(audio @ Waa.T + visual @ Wva.T + b)

with
    Wvo = Wo @ Wi[2E:]             bvo = Wo @ bi[2E:] + bo
    Wfv = Wf[:, :E] @ Wvo          Wfa = Wf[:, E:] @ Wvo
    Waa = Wfa @ Wa                 Wva = Wfv @ Wv
    b   = Wfa @ ba + Wfv @ bv + (Wf[:, :E] + Wf[:, E:]) @ bvo + bf

Weight composition is done on host in float64 (cheap: ~15 GFLOP), the big
GEMM (32768 x 4096 @ 4096 x 1024, 275 GFLOP) runs on 8 NeuronCores, batch
sharded (pure data parallel per the sharding hint).

Mixed-precision contraction: the PE runs bf16 at 1 cyc/row and fp8-e4m3
DoubleRow at 0.5 cyc/row (contracting 256 rows per instruction). Putting
the last K8=1024 of the 4096 contraction rows in fp8 cuts PE time 12.5%
while the measured end-to-end max-rel error stays at 1.75e-2 vs the 2e-2
gate (bf16-only is 2.0e-3). The fp8 operands are reciprocally pre-scaled
on host (x/2^5, w*2^5) so their products land at the correct scale and
accumulate into the same PSUM group as the bf16 part; e4m3 cannot
represent the raw w ~ 1e-3 values (subnormal cutoff 2^-6) without this.

Device layout per core:
    xtb [KB=3072, BC=4096] bf16 - activations, feature-major
    xt8 [K8=1024, BC=4096] f8e4 - last K-slice, pre-scaled 2^-5
    wtb [KB, E=1024]       bf16 - composed weight, feature-major
    wt8 [K8, E]            f8e4 - pre-scaled 2^+5 (replicated)
    bias[P=128,  E]        f32  - row-replicated bias
    out [BC, E]            f32  - natural layout

PSUM tile [128 batch, 512 outfeat] (one bank; matmul cannot cross a PSUM
bank boundary): stationary = x subtile, moving = w tile. Per batch tile:
24 bf16 k-steps then 4 DoubleRow steps (lhsT [128,2,128], rhs [128,2,512])
accumulate, then DVE adds bias PSUM->SBUF, ScalarE applies Relu, DMA out.

DMA preamble is ordered just-in-time as (xch[k], wt[k]) pairs so the PE
starts after ~0.4 MB instead of after the whole weight set; bf16 per-k
demand (384 KB / 1.2 us) stays under the PE k-step time (1.7 us) so the
first sweep never starves, and the fp8 chunks ride in the slack before
the sweep reaches them. The final batch tiles shrink (512x7, 256, 128,
128) because the last tile's PSUM drain + store-out cannot overlap any
compute.
"""

import os
import sys

import numpy as np

sys.path.insert(0, "/opt/trn_rl_repo")

import ml_dtypes

import concourse.bacc as bacc
import concourse.mybir as mybir
import concourse.tile as tile
from concourse.bass_utils import run_bass_kernel_spmd


def _ensure_ntff_hook():
    """Register the axon NTFF profile hook if boot() couldn't (the image's
    antenv may lack axon_hooks; without this, trace=True silently degrades)."""
    try:
        import antenv.axon_hooks as ah
    except ImportError:
        import types

        import antenv

        ah = types.ModuleType("antenv.axon_hooks")
        ah._HOOK = None
        ah.set_axon_ntff_profile_hook = lambda h: setattr(ah, "_HOOK", h)
        ah.get_axon_ntff_profile_hook = lambda: ah._HOOK
        sys.modules["antenv.axon_hooks"] = ah
        antenv.axon_hooks = ah
    try:
        if ah.get_axon_ntff_profile_hook() is None:
            from trn_agent_boot.trn_boot import _ntff_profile_via_ctypes

            ah.set_axon_ntff_profile_hook(
                _ntff_profile_via_ctypes("/opt/axon/libaxon_pjrt.so")
            )
    except Exception:
        pass


_ensure_ntff_hook()

N_CORES = 8
B = 32768
BC = B // N_CORES  # 4096 batch rows per core
K = 4096           # 2048 audio + 2048 visual features
E = 1024
P = 128

MODE = os.environ.get("KMM_MODE", "mix8")  # "mix8" | "bf16"
K8 = 1024 if MODE == "mix8" else 0  # fp8 contraction rows (last K-slice)
A8 = 5                              # reciprocal power-of-2 operand scale
KB = K - K8
KBT = KB // P      # bf16 contraction tiles
J8 = K8 // (2 * P) # fp8 DoubleRow steps (256 rows each)
NB = 512           # main batch tile
# Two 256-row final tiles shrink the end-of-kernel drain tail. No smaller:
# a tile costs ~30 DMA issues (~650 ns each on the issuing engine) and a
# 128-row tile's 12 us sweep can't cover that, so the PE starves.
TILES = [NB] * 7 + [256, 256]
assert sum(TILES) == BC
M2 = E // NB       # 2 outfeat halves (PSUM free dim limit: one 2KB bank)

_NC_CACHE = {}
LAST_RESULTS = None  # stashed BassKernelResults for test.py introspection


def _build_nc(mode):
    bf16 = mybir.dt.bfloat16
    f8 = mybir.dt.float8e4
    f32 = mybir.dt.float32

    nc = bacc.Bacc("TRN2", debug=False, target_bir_lowering=False)
    xtb = nc.dram_tensor("xtb", [KB, BC], bf16, kind="ExternalInput").ap()
    wtb = nc.dram_tensor("wtb", [KB, E], bf16, kind="ExternalInput").ap()
    if K8:
        xt8 = nc.dram_tensor("xt8", [K8, BC], f8, kind="ExternalInput").ap()
        wt8 = nc.dram_tensor("wt8", [K8, E], f8, kind="ExternalInput").ap()
    bias = nc.dram_tensor("bias", [P, E], f32, kind="ExternalInput").ap()
    out = nc.dram_tensor("out", [BC, E], f32, kind="ExternalOutput").ap()

    with tile.TileContext(nc) as tc:
        with (
            tc.tile_pool(name="wpool", bufs=1) as wpool,
            tc.tile_pool(name="xpool", bufs=12) as xpool,
            tc.tile_pool(name="x8pool", bufs=6) as x8pool,
            tc.tile_pool(name="opool", bufs=8) as opool,
            tc.tile_pool(name="pspool", bufs=8, space="PSUM") as pspool,
        ):
            # DMA arrival order == emission order per queue. All input
            # streams issue from the Sync queue in just-in-time order for
            # batch tile 0's k-sweep (the GpSimd queue was measured slower
            # to issue, starving the sweep); output stores issue from the
            # Scalar queue so ~16 issues/tile (~650 ns each) stay off the
            # Sync stream.
            wtb_sb = wpool.tile([P, KBT, E], bf16)
            wtb_r = wtb.rearrange("(ko ki) e -> ki ko e", ki=P)
            if K8:
                wt8_sb = wpool.tile([P, 2 * J8, E], f8)
                wt8_r = wt8.rearrange("(ko ki) e -> ki ko e", ki=P)
                xt8_r = xt8.rearrange("(c ki) b -> ki c b", ki=P)
            bias_sb = wpool.tile([P, E], f32)

            xch0 = {}
            for k in range(8):
                xch = xpool.tile([P, NB], bf16, tag="xch")
                if k == 0:
                    # Split the first pair so the first matmul's operands
                    # (stationary xch[:, 0:128], moving wt[:, 0, 0:512])
                    # land as early as possible.
                    nc.sync.dma_start(xch[:, 0:P], xtb[0:P, 0:P])
                    nc.sync.dma_start(wtb_sb[:, 0, 0:NB], wtb_r[:, 0, 0:NB])
                    nc.sync.dma_start(xch[:, P:NB], xtb[0:P, P:NB])
                    nc.sync.dma_start(wtb_sb[:, 0, NB:E], wtb_r[:, 0, NB:E])
                else:
                    nc.sync.dma_start(xch, xtb[k * P : (k + 1) * P, 0:NB])
                    nc.sync.dma_start(wtb_sb[:, k], wtb_r[:, k])
                xch0[k] = xch
            for k in range(8, KBT):
                if k % 4 == 0:
                    nc.sync.dma_start(
                        wtb_sb[:, k : k + 4], wtb_r[:, k : k + 4]
                    )
                xch = xpool.tile([P, NB], bf16, tag="xch")
                nc.sync.dma_start(xch, xtb[k * P : (k + 1) * P, 0:NB])
                xch0[k] = xch
                if k == 11:
                    nc.scalar.dma_start(bias_sb, bias)
            xch80 = {}
            for j in range(J8):
                # fp8 chunks ride in the first sweep's DMA slack (the PE is
                # still ~17 us away from needing them when these are issued).
                xch8 = x8pool.tile([P, 2, NB], f8, tag="xch8")
                nc.sync.dma_start(xch8, xt8_r[:, 2 * j : 2 * j + 2, 0:NB])
                xch80[j] = xch8
                nc.sync.dma_start(
                    wt8_sb[:, 2 * j : 2 * j + 2], wt8_r[:, 2 * j : 2 * j + 2]
                )

            off = 0
            for n, nb in enumerate(TILES):
                b4 = nb // P
                psums = [
                    pspool.tile([P, NB], f32, tag="ps", name=f"ps_{n}_{j}")
                    for j in range(b4 * M2)
                ]
                for k in range(KBT):
                    if n == 0:
                        xch = xch0[k]
                    else:
                        xch = xpool.tile([P, nb], bf16, tag=f"xch{nb}")
                        nc.sync.dma_start(
                            xch, xtb[k * P : (k + 1) * P, off : off + nb]
                        )
                    for b in range(b4):
                        for m in range(M2):
                            nc.tensor.matmul(
                                psums[b * M2 + m],
                                lhsT=xch[:, b * P : (b + 1) * P],
                                rhs=wtb_sb[:, k, m * NB : (m + 1) * NB],
                                start=(k == 0),
                                stop=False,
                            )
                for j in range(J8):
                    if n == 0:
                        xch8 = xch80[j]
                    else:
                        xch8 = x8pool.tile([P, 2, nb], f8, tag=f"xch8{nb}")
                        nc.sync.dma_start(
                            xch8, xt8_r[:, 2 * j : 2 * j + 2, off : off + nb]
                        )
                    for b in range(b4):
                        for m in range(M2):
                            nc.tensor.matmul(
                                psums[b * M2 + m],
                                lhsT=xch8[:, :, b * P : (b + 1) * P],
                                rhs=wt8_sb[:, 2 * j : 2 * j + 2, m * NB : (m + 1) * NB],
                                start=False,
                                stop=(j == J8 - 1),
                                perf_mode=mybir.MatmulPerfMode.DoubleRow,
                            )
                for b in range(b4):
                    for m in range(M2):
                        ps = psums[b * M2 + m]
                        osb = opool.tile([P, NB], f32, tag="osb")
                        nc.vector.tensor_add(
                            out=osb,
                            in0=ps,
                            in1=bias_sb[:, m * NB : (m + 1) * NB],
                        )
                        nc.scalar.activation(
                            osb, osb, mybir.ActivationFunctionType.Relu
                        )
                        nc.scalar.dma_start(
                            out[
                                off + b * P : off + (b + 1) * P,
                                m * NB : (m + 1) * NB,
                            ],
                            osb,
                        )
                off += nb

    nc.compile()
    return nc


def _get_nc(mode):
    if mode not in _NC_CACHE:
        _NC_CACHE[mode] = _build_nc(mode)
    return _NC_CACHE[mode]


def _compose_weights(Wa, ba, Wv, bv, Wi, bi, Wo, bo, Wf, bf):
    f6 = lambda x: np.asarray(x, dtype=np.float64)
    Wvo = f6(Wo) @ f6(Wi[2 * E :])
    bvo = f6(Wo) @ f6(bi[2 * E :]) + f6(bo)
    Wf1, Wf2 = f6(Wf[:, :E]), f6(Wf[:, E:])
    Wfv = Wf1 @ Wvo  # applied to visual_e for audio_att
    Wfa = Wf2 @ Wvo  # applied to audio_e for visual_att
    Waa = Wfa @ f6(Wa)  # [E, 2048] applied to audio
    Wva = Wfv @ f6(Wv)  # [E, 2048] applied to visual
    b = Wfa @ f6(ba) + Wfv @ f6(bv) + (Wf1 + Wf2) @ bvo + f6(bf)
    wt = np.concatenate([Waa, Wva], axis=1).T  # [K, E] float64
    return wt, b.astype(np.float32)


def kernel(audio, visual, Wa, ba, Wv, bv, Wi, bi, Wo, bo, Wf, bf):
    global LAST_RESULTS
    wt, bias = _compose_weights(Wa, ba, Wv, bv, Wi, bi, Wo, bo, Wf, bf)
    bias_bc = np.ascontiguousarray(np.broadcast_to(bias, (P, E)), np.float32)

    bfdt = ml_dtypes.bfloat16
    f8 = ml_dtypes.float8_e4m3
    wtb = np.ascontiguousarray(wt[:KB]).astype(bfdt)
    if K8:
        wt8 = np.ascontiguousarray(wt[KB:] * 2.0**A8).astype(f8)
    audio = np.asarray(audio, dtype=np.float32)
    visual = np.asarray(visual, dtype=np.float32)

    in_maps = []
    for c in range(N_CORES):
        rows = slice(c * BC, (c + 1) * BC)
        at = audio[rows].T  # [2048, BC]
        vt = visual[rows].T  # [2048, BC]
        xtb_c = np.empty((KB, BC), bfdt)
        xtb_c[:2048] = at
        xtb_c[2048:] = vt[: KB - 2048]
        m = {"xtb": xtb_c, "wtb": wtb, "bias": bias_bc}
        if K8:
            m["xt8"] = (vt[KB - 2048 :] * 2.0**-A8).astype(f8)
            m["wt8"] = wt8
        in_maps.append(m)

    nc = _get_nc(MODE)
    trace = os.environ.get("KMM_TRACE", "0") == "1"
    kwargs = {}
    if os.environ.get("KMM_TRACE_ALL", "0") == "1":
        kwargs["trace_cores"] = list(range(N_CORES))
    res = run_bass_kernel_spmd(
        nc, in_maps, core_ids=list(range(N_CORES)), trace=trace, **kwargs
    )
    LAST_RESULTS = res
    out = np.concatenate([r["out"] for r in res.results], axis=0)
    return np.ascontiguousarray(out, dtype=np.float32)
